# revision 39
# baseline (speedup 1.0000x reference)
"""Linear-attention Trainium2 kernel (8 NeuronCores, SPMD).

Sharding: batch (4) x head-group (2). Core i handles batch i//2, heads
[8*(i%2), 8*(i%2)+8). Each core computes its partial output through Wo;
the host sums the two partials per batch and adds bo.

Precision scheme (f32 PSUM accumulation everywhere):
  - K projection: 1-term fp8 e4m3 DoubleRow (x8 . wk8)
  - Q, V, output projections: 3-term split-fp8 DoubleRow
      a.w ~= a8.w8 + ar8.w8 + a8.wr8   (ar8/wr8 = fp8 residuals)
    which costs 0.75x one bf16 GEMM and is ~2x more accurate than bf16.
  - Weights are pre-scaled by 16 on the host so their fp8 residuals clear
    the e4m3 denormal floor; the scale is compensated for free in the
    ACT exp (scale=1/16), the out^T copy, and the final y copy.
  - Attention intermediates (ek, vn, eqn, kvsb) are bf16.

Per-core dataflow:
  phase 1 (K/V/KV): per 128-row sequence tile
    K = x8 @ wk8 (DR), ek = exp(K/16) [* exp(bk)]
    rk = 1/rowsum_per_head(ek)
    vn = (x.wv split-fp8 [+ 16*bv]) * rk          [128, 512], 16x scale
    KV_h += ek_h^T @ vn_h  (exact block placement, one memset bank,
                            start=False accumulation, emitted 2 tiles late)
  phase 2 (Q/out/y): per 512-column chunk, interleaved so chunk c's Q
    matmuls cover chunk c-1's out^T and chunk c-2's y-store latencies:
    Q^T = wq.x split-fp8 (DR), eq = exp(Q^T/16 + bq)
    sqb = blockdiag-ones @ eq -> per-head colsums broadcast
    eqn = eq * (1/sqb)
    ops = KV^T-contract eqn;  otc8 = fp8(ops/16), otr8 = fp8 residual
    y = otc.wo split-fp8 (DR), ysb = yps/16, DMA per 512-col half
"""

import numpy as np

B, S, DM, H = 4, 4096, 1024, 16
HD = 64
GROUPS = 2
DLOC = DM // GROUPS   # 512 channels per core
HLOC = H // GROUPS    # 8 heads per core
NCORES = B * GROUPS   # 8
SC = 512              # sequence chunk
NTF = DM // 128       # 8 contraction tiles over d_model
NPAIR = DLOC // 128   # 4 pair-tiles (2 heads each)
WS = 16.0             # host-side weight scale (power of 2)


def make_sel():
    """[128,128] block 'ones' that broadcast per-head colsums: out[q,s] =
    sum_p sel[p,q] eq[p,s] with sel[p,q]=1 iff p,q in the same 64-half."""
    sel = np.zeros((128, 128), np.float32)
    sel[:64, :64] = 1.0
    sel[64:, 64:] = 1.0
    return sel


def build_bass(S_=S, n_devices=NCORES, repeat=1, dbg=False, bias_free=True):
    from contextlib import ExitStack
    import concourse.bass as bass
    import concourse.bacc as bacc
    import concourse.mybir as mybir
    import concourse.tile as tile

    f32 = mybir.dt.float32
    bf16 = mybir.dt.bfloat16
    fp8 = mybir.dt.float8e4
    Exp = mybir.ActivationFunctionType.Exp
    X = mybir.AxisListType.X
    Mult = mybir.AluOpType.mult
    Add = mybir.AluOpType.add
    Sub = mybir.AluOpType.subtract
    DR = mybir.MatmulPerfMode.DoubleRow
    RS = 1.0 / WS

    NCH = S_ // SC        # sequence chunks
    NST = S_ // 128       # sequence tiles

    nc = bacc.Bacc("TRN2", target_bir_lowering=False, debug=False,
                   num_devices=n_devices)
    x8 = nc.dram_tensor("x8", [128, NTF, S_], fp8, kind="ExternalInput").ap()
    xr8 = nc.dram_tensor("xr8", [128, NTF, S_], fp8,
                         kind="ExternalInput").ap()
    wq8 = nc.dram_tensor("wq8", [128, NTF, DLOC], fp8,
                         kind="ExternalInput").ap()
    wqr8 = nc.dram_tensor("wqr8", [128, NTF, DLOC], fp8,
                          kind="ExternalInput").ap()
    wk8 = nc.dram_tensor("wk8", [128, NTF, DLOC], fp8,
                         kind="ExternalInput").ap()
    wv8 = nc.dram_tensor("wv8", [128, NTF, DLOC], fp8,
                         kind="ExternalInput").ap()
    wvr8 = nc.dram_tensor("wvr8", [128, NTF, DLOC], fp8,
                          kind="ExternalInput").ap()
    wo8 = nc.dram_tensor("wo8", [128, NPAIR, DM], fp8,
                         kind="ExternalInput").ap()
    wor8 = nc.dram_tensor("wor8", [128, NPAIR, DM], fp8,
                          kind="ExternalInput").ap()
    bqT = nc.dram_tensor("bqT", [128, NPAIR], f32, kind="ExternalInput").ap()
    ekb = nc.dram_tensor("ekb", [128, DLOC], bf16, kind="ExternalInput").ap()
    bvb = nc.dram_tensor("bvb", [128, DLOC], bf16, kind="ExternalInput").ap()
    sel = nc.dram_tensor("sel", [128, 128], bf16, kind="ExternalInput").ap()
    y = nc.dram_tensor("y", [S_, DM], f32, kind="ExternalOutput").ap()
    if dbg:
        d_kv = nc.dram_tensor("d_kv", [128, 512], f32,
                              kind="ExternalOutput").ap()

    def body(tc):
        ctx = ExitStack()
        with ctx:
            cons = ctx.enter_context(tc.tile_pool(name="cons", bufs=1))
            xpool = ctx.enter_context(tc.tile_pool(name="xp", bufs=1))
            wpool = ctx.enter_context(tc.tile_pool(name="wp", bufs=1))
            kvsbp = ctx.enter_context(tc.tile_pool(name="kvsb", bufs=1))

            wscr = cons.tile([128, SC], bf16)
            nc.vector.memset(wscr, 0.0)

            # ---- streaming loads, ordered by first use ----
            x8sb = xpool.tile([128, NTF, S_], fp8, tag="x8")
            xr8sb = xpool.tile([128, NTF, S_], fp8, tag="xr8")
            wk_sb = wpool.tile([128, NTF, DLOC], fp8, tag="wk")
            wv_sb = wpool.tile([128, NTF, DLOC], fp8, tag="wv")
            wvr_sb = wpool.tile([128, NTF, DLOC], fp8, tag="wvr")
            wq_sb = wpool.tile([128, NTF, DLOC], fp8, tag="wq")
            wqr_sb = wpool.tile([128, NTF, DLOC], fp8, tag="wqr")
            wo_sb = wpool.tile([128, NPAIR, DM], fp8, tag="wo")
            wor_sb = wpool.tile([128, NPAIR, DM], fp8, tag="wor")
            h1, h2 = slice(0, 4), slice(4, 8)
            nc.sync.dma_start(out=wk_sb[:, h1, :], in_=wk8[:, h1, :])
            nc.sync.dma_start(out=x8sb[:, h1, 0:SC], in_=x8[:, h1, 0:SC])
            nc.sync.dma_start(out=wk_sb[:, h2, :], in_=wk8[:, h2, :])
            nc.sync.dma_start(out=x8sb[:, h2, 0:SC], in_=x8[:, h2, 0:SC])
            nc.sync.dma_start(out=wv_sb[:, h1, :], in_=wv8[:, h1, :])
            nc.sync.dma_start(out=wv_sb[:, h2, :], in_=wv8[:, h2, :])
            nc.sync.dma_start(out=xr8sb[:, :, 0:SC], in_=xr8[:, :, 0:SC])
            nc.sync.dma_start(out=wvr_sb, in_=wvr8)
            nc.sync.dma_start(out=x8sb[:, :, SC:2 * SC],
                              in_=x8[:, :, SC:2 * SC])
            nc.sync.dma_start(out=xr8sb[:, :, SC:2 * SC],
                              in_=xr8[:, :, SC:2 * SC])
            sel_sb = cons.tile([128, 128], bf16)
            nc.sync.dma_start(out=sel_sb, in_=sel)
            bqT_sb = cons.tile([128, NPAIR], f32)
            nc.sync.dma_start(out=bqT_sb, in_=bqT)
            if not bias_free:
                ekb_sb = cons.tile([128, DLOC], bf16)
                nc.sync.dma_start(out=ekb_sb, in_=ekb)
                bvb_sb = cons.tile([128, DLOC], bf16)
                nc.sync.dma_start(out=bvb_sb, in_=bvb)
            for c in range(2, NCH):
                nc.sync.dma_start(out=x8sb[:, :, c * SC:(c + 1) * SC],
                                  in_=x8[:, :, c * SC:(c + 1) * SC])
                nc.sync.dma_start(out=xr8sb[:, :, c * SC:(c + 1) * SC],
                                  in_=xr8[:, :, c * SC:(c + 1) * SC])
            nc.sync.dma_start(out=wq_sb, in_=wq8)
            nc.sync.dma_start(out=wqr_sb, in_=wqr8)
            nc.sync.dma_start(out=wo_sb, in_=wo8)
            nc.sync.dma_start(out=wor_sb, in_=wor8)

            kvsb = kvsbp.tile([128, 512], bf16)
            VT3 = [(x8sb, wv_sb), (xr8sb, wv_sb), (x8sb, wvr_sb)]
            QT3 = [(wq_sb, x8sb), (wq_sb, xr8sb), (wqr_sb, x8sb)]

            # ---------------- phase 1: K / V / KV ----------------
            with ExitStack() as p1:
                pkv = p1.enter_context(
                    tc.tile_pool(name="pkv", bufs=3, space="PSUM"))
                kvp = p1.enter_context(
                    tc.tile_pool(name="kvps", bufs=1, space="PSUM"))
                ekpool = p1.enter_context(tc.tile_pool(name="ek", bufs=6))
                vnpool = p1.enter_context(tc.tile_pool(name="vn", bufs=6))
                smpool = p1.enter_context(tc.tile_pool(name="sm", bufs=6))

                kv = kvp.tile([128, 512], f32)
                # real HW start=True clears only the written region (not the
                # whole 2KB row the sim models), so zero the bank explicitly
                # and accumulate with start=False throughout.
                nc.vector.memset(kv, 0.0)
                pendings = []

                def emit_kv(ek, vn, st):
                    for h in range(HLOC):
                        p_, hf = h // 2, 64 * (h % 2)
                        nc.tensor.matmul(
                            kv[hf:hf + 64, p_ * 128 + hf:p_ * 128 + hf + 64],
                            ek[:, h * HD:(h + 1) * HD],
                            vn[:, h * HD:(h + 1) * HD],
                            start=False,
                            stop=(st == NST - 1),
                            skip_group_check=True)

                def drain_kv(keep=0):
                    if len(pendings) > keep:
                        emit_kv(*pendings.pop(0))

                def emit_k_mms(kps, st):
                    for tfp in range(NTF // 2):
                        nc.tensor.matmul(
                            kps,
                            x8sb[:, 2 * tfp:2 * tfp + 2,
                                 st * 128:(st + 1) * 128],
                            wk_sb[:, 2 * tfp:2 * tfp + 2, :],
                            start=(tfp == 0), stop=(tfp == NTF // 2 - 1),
                            perf_mode=DR)

                def emit_v_mms(vps, st, term, first, last):
                    a, w = VT3[term]
                    for tfp in range(NTF // 2):
                        nc.tensor.matmul(
                            vps,
                            a[:, 2 * tfp:2 * tfp + 2,
                              st * 128:(st + 1) * 128],
                            w[:, 2 * tfp:2 * tfp + 2, :],
                            start=(first and tfp == 0),
                            stop=(last and tfp == NTF // 2 - 1),
                            perf_mode=DR)

                def k_chain(kps):
                    """exp(K/16) + per-head rowsum reciprocal."""
                    ek0 = ekpool.tile([128, DLOC], bf16, tag="ek0")
                    nc.scalar.activation(ek0, kps, Exp, scale=RS)
                    with nc.allow_low_precision(reason="bf16 attn weights"):
                        if bias_free:
                            ek = ek0
                        else:
                            ek = ekpool.tile([128, DLOC], bf16, tag="ek")
                            nc.vector.tensor_tensor(out=ek, in0=ek0,
                                                    in1=ekb_sb, op=Mult)
                        sk = smpool.tile([128, HLOC], bf16, tag="sk")
                        nc.vector.reduce_sum(
                            sk, ek.rearrange("p (h e) -> p h e", e=HD),
                            axis=X)
                        rk = smpool.tile([128, HLOC], bf16, tag="rk")
                        nc.vector.reciprocal(rk, sk)
                    return ek, rk

                def v_chain(st, ek, rk, vps):
                    """bias + normalize one V tile, queue its KV update."""
                    vn = vnpool.tile([128, DLOC], bf16, tag="vn")
                    rkb = bass.AP(
                        tensor=rk.tensor, offset=rk.offset,
                        ap=[list(rk.ap[0]), [1, HLOC], [0, HD]])
                    with nc.allow_low_precision(reason="bf16 attn weights"):
                        if bias_free:
                            v1 = vps
                        else:
                            v1 = vnpool.tile([128, DLOC], bf16, tag="v1")
                            nc.vector.tensor_tensor(out=v1, in0=vps,
                                                    in1=bvb_sb, op=Add)
                        nc.vector.tensor_tensor(
                            out=vn.rearrange("p (h e) -> p h e", e=HD),
                            in0=v1.rearrange("p (h e) -> p h e", e=HD),
                            in1=rkb, op=Mult)
                    pendings.append((ek, vn, st))

                # --- chunk 0: term/tf-major so the PE can consume each
                # weight/x slice the moment it lands ---
                kps0 = []
                for t in range(4):
                    k0 = pkv.tile([128, DLOC], f32, tag=f"c0_{t}", bufs=1)
                    kps0.append(k0)
                # PE warmup during the initial DMA wait (ramps the clock);
                # overwritten by the first real matmul (start=True).
                for _ in range(4):
                    nc.tensor.matmul(kps0[0], wscr[:, 0:128], wscr,
                                     start=True, stop=True)
                for tfp in range(NTF // 2):
                    for t in range(4):
                        nc.tensor.matmul(
                            kps0[t],
                            x8sb[:, 2 * tfp:2 * tfp + 2,
                                 t * 128:(t + 1) * 128],
                            wk_sb[:, 2 * tfp:2 * tfp + 2, :],
                            start=(tfp == 0), stop=(tfp == NTF // 2 - 1),
                            perf_mode=DR)
                chains0 = [k_chain(kps0[t]) for t in range(4)]
                vps0 = []
                for t in range(4):
                    v0 = pkv.tile([128, DLOC], f32, tag=f"c0_{t}", bufs=1)
                    vps0.append(v0)
                for term in range(3):
                    a, w = VT3[term]
                    for tfp in range(NTF // 2):
                        for t in range(4):
                            nc.tensor.matmul(
                                vps0[t],
                                a[:, 2 * tfp:2 * tfp + 2,
                                  t * 128:(t + 1) * 128],
                                w[:, 2 * tfp:2 * tfp + 2, :],
                                start=(term == 0 and tfp == 0),
                                stop=(term == 2 and tfp == NTF // 2 - 1),
                                perf_mode=DR)
                for t in range(4):
                    v_chain(t, chains0[t][0], chains0[t][1], vps0[t])

                # --- chunks 1..7: tile-major, kv emission trails by 2 ---
                for st in range(4, NST):
                    kps = pkv.tile([128, DLOC], f32, tag="pkv")
                    emit_k_mms(kps, st)
                    drain_kv()
                    ek, rk = k_chain(kps)
                    vps = pkv.tile([128, DLOC], f32, tag="pkv")
                    for term in range(3):
                        emit_v_mms(vps, st, term, term == 0, term == 2)
                    v_chain(st, ek, rk, vps)
                    drain_kv(keep=2)

                while pendings:
                    drain_kv()
                nc.scalar.copy(kvsb, kv)
                if dbg:
                    kvf = ekpool.tile([128, 512], f32, tag="kvf")
                    nc.vector.tensor_copy(kvf, kv)
                    nc.sync.dma_start(out=d_kv, in_=kvf)

            # ---------------- phase 2: Q / out / y ----------------
            with ExitStack() as p2:
                eqpool = p2.enter_context(tc.tile_pool(name="eq", bufs=4))
                eqnpool = p2.enter_context(tc.tile_pool(name="eqn", bufs=2))
                rqpool = p2.enter_context(tc.tile_pool(name="rq", bufs=4))
                otpool = p2.enter_context(tc.tile_pool(name="ot", bufs=2))
                ysbpool = p2.enter_context(tc.tile_pool(name="ysb", bufs=4))
                qpsp = p2.enter_context(
                    tc.tile_pool(name="qps", bufs=2, space="PSUM"))
                sqpsp = p2.enter_context(
                    tc.tile_pool(name="sqps", bufs=2, space="PSUM"))
                opsp = p2.enter_context(
                    tc.tile_pool(name="ops", bufs=2, space="PSUM"))
                ypsp = p2.enter_context(
                    tc.tile_pool(name="yps", bufs=2, space="PSUM"))

                def emit_q(c, dt, eqn):
                    """12 split-fp8 Q matmuls + exp for pair-tile dt; returns
                    a closure finishing the normalization (emitted later)."""
                    qps = qpsp.tile([128, SC], f32, tag="q")
                    for term in range(3):
                        wsrc, asrc = QT3[term]
                        for tfp in range(NTF // 2):
                            nc.tensor.matmul(
                                qps,
                                wsrc[:, 2 * tfp:2 * tfp + 2,
                                     dt * 128:(dt + 1) * 128],
                                asrc[:, 2 * tfp:2 * tfp + 2,
                                     c * SC:(c + 1) * SC],
                                start=(term == 0 and tfp == 0),
                                stop=(term == 2 and tfp == NTF // 2 - 1),
                                perf_mode=DR)
                    eq = eqpool.tile([128, SC], bf16, tag="eq")
                    nc.scalar.activation(eq, qps, Exp,
                                         bias=bqT_sb[:, dt:dt + 1], scale=RS)

                    def fin():
                        sq = sqpsp.tile([128, SC], f32, tag="sq")
                        nc.tensor.matmul(sq, sel_sb, eq, start=True, stop=True)
                        rq = rqpool.tile([128, SC], bf16, tag="rq")
                        with nc.allow_low_precision(reason="bf16 attn"):
                            nc.vector.reciprocal(rq, sq)
                            nc.vector.tensor_tensor(
                                out=eqn[:, dt, :], in0=eq, in1=rq, op=Mult)
                    return fin

                def emit_ops(eqn, otc8, otr8):
                    for dt in range(NPAIR):
                        ops = opsp.tile([128, SC], f32, tag="ops")
                        nc.tensor.matmul(ops,
                                         kvsb[:, dt * 128:(dt + 1) * 128],
                                         eqn[:, dt, :], start=True, stop=True)
                        with nc.allow_low_precision(reason="fp8 split"):
                            nc.scalar.mul(otc8[:, dt, :], ops, RS)
                            nc.vector.scalar_tensor_tensor(
                                out=otr8[:, dt, :], in0=ops, scalar=RS,
                                in1=otc8[:, dt, :], op0=Mult, op1=Sub)

                def emit_y(otc8, otr8, c, t):
                    row = (c * 4 + t) * 128
                    for jh in range(2):
                        yps = ypsp.tile([128, 512], f32, tag="yps")
                        first = True
                        for lsrc, rsrc in ((otc8, wo_sb), (otr8, wo_sb),
                                           (otc8, wor_sb)):
                            for ctp in range(NPAIR // 2):
                                nc.tensor.matmul(
                                    yps,
                                    lsrc[:, 2 * ctp:2 * ctp + 2,
                                         t * 128:(t + 1) * 128],
                                    rsrc[:, 2 * ctp:2 * ctp + 2,
                                         jh * 512:(jh + 1) * 512],
                                    start=first,
                                    stop=(lsrc is otc8 and rsrc is wor_sb
                                          and ctp == NPAIR // 2 - 1),
                                    perf_mode=DR)
                                first = False
                        ysb = ysbpool.tile([128, 512], f32, tag="ysb")
                        if jh == 0:
                            nc.scalar.mul(ysb, yps, RS)
                        else:
                            nc.vector.tensor_scalar_mul(ysb, yps, RS)
                        nc.sync.dma_start(
                            out=y[row:row + 128, jh * 512:(jh + 1) * 512],
                            in_=ysb)

                yq = []

                def drain_y():
                    if yq:
                        otc8_, otr8_, c_, t_ = yq.pop(0)
                        emit_y(otc8_, otr8_, c_, t_)

                prev = None   # (eqn, c) of previous chunk
                for c in range(NCH):
                    eqn = eqnpool.tile([128, NPAIR, SC], bf16, tag="eqn")
                    fins = [emit_q(c, 0, eqn)]
                    if prev is not None:
                        otc8 = otpool.tile([128, NPAIR, SC], fp8, tag="otc")
                        otr8 = otpool.tile([128, NPAIR, SC], fp8, tag="otr")
                        emit_ops(prev[0], otc8, otr8)
                    fins.append(emit_q(c, 1, eqn))
                    fins.append(emit_q(c, 2, eqn))
                    fins.pop(0)()
                    drain_y()
                    fins.append(emit_q(c, 3, eqn))
                    fins.pop(0)()
                    drain_y()
                    fins.pop(0)()
                    drain_y()
                    fins.pop(0)()
                    drain_y()
                    if prev is not None:
                        for t in range(4):
                            yq.append((otc8, otr8, prev[1], t))
                    prev = (eqn, c)
                # epilogue: last chunk's out/y, interleaved with deferred y
                otc8 = otpool.tile([128, NPAIR, SC], fp8, tag="otc")
                otr8 = otpool.tile([128, NPAIR, SC], fp8, tag="otr")
                drain_y()
                emit_ops(prev[0], otc8, otr8)
                while yq:
                    drain_y()
                for t in range(4):
                    emit_y(otc8, otr8, prev[1], t)

    with tile.TileContext(nc) as tc:
        if repeat == 1:
            body(tc)
        else:
            for _ in range(repeat):
                body(tc)
    nc.compile()
    return nc


def _split8(a, f8):
    """fp8 value + fp8 residual of an array."""
    a8 = a.astype(f8)
    r8 = (a - a8.astype(a.dtype)).astype(f8)
    return np.ascontiguousarray(a8), np.ascontiguousarray(r8)


def shard_inputs(x, Wq, bq, Wk, bk, Wv, bv, Wo, S_=S):
    import ml_dtypes
    bf16 = ml_dtypes.bfloat16
    f8 = ml_dtypes.float8_e4m3
    f = np.float32
    sel = make_sel().astype(bf16)
    x = np.asarray(x, dtype=f)
    in_maps = []
    xt_cache = {}
    for core in range(NCORES):
        b, g = core // GROUPS, core % GROUPS
        sl = slice(g * DLOC, (g + 1) * DLOC)
        if b not in xt_cache:
            xr = np.ascontiguousarray(
                x[b, :S_, :].T.reshape(NTF, 128, S_).transpose(1, 0, 2))
            xt_cache[b] = _split8(xr, f8)
        wq_ = np.asarray(Wq, f)[:, sl].reshape(NTF, 128, DLOC).transpose(
            1, 0, 2) * f(WS)
        wk_ = np.asarray(Wk, f)[:, sl].reshape(NTF, 128, DLOC).transpose(
            1, 0, 2) * f(WS)
        wv_ = np.asarray(Wv, f)[:, sl].reshape(NTF, 128, DLOC).transpose(
            1, 0, 2) * f(WS)
        wo_ = np.asarray(Wo, f)[sl, :].reshape(NPAIR, 128, DM).transpose(
            1, 0, 2) * f(WS)
        wq8_, wqr8_ = _split8(wq_, f8)
        wv8_, wvr8_ = _split8(wv_, f8)
        wo8_, wor8_ = _split8(wo_, f8)
        bqT_ = np.ascontiguousarray(
            np.asarray(bq, f)[sl].reshape(NPAIR, 128).T).astype(f)
        ekb_ = np.broadcast_to(np.exp(np.asarray(bk, f)[sl]),
                               (128, DLOC)).astype(bf16)
        bvb_ = np.broadcast_to(np.asarray(bv, f)[sl] * f(WS),
                               (128, DLOC)).astype(bf16)
        in_maps.append({
            "x8": xt_cache[b][0], "xr8": xt_cache[b][1],
            "wq8": wq8_, "wqr8": wqr8_,
            "wk8": np.ascontiguousarray(wk_.astype(f8)),
            "wv8": wv8_, "wvr8": wvr8_, "wo8": wo8_, "wor8": wor8_,
            "bqT": bqT_, "ekb": np.ascontiguousarray(ekb_),
            "bvb": np.ascontiguousarray(bvb_), "sel": sel,
        })
    return in_maps


_NC_CACHE = {}


def _get_nc(bias_free=True):
    key = f"nc{bias_free}"
    if key not in _NC_CACHE:
        _NC_CACHE[key] = build_bass(bias_free=bias_free)
    return _NC_CACHE[key]


def kernel(x, Wq, bq, Wk, bk, Wv, bv, Wo, bo):
    from concourse.bass_utils import run_bass_kernel_spmd
    bias_free = not (np.any(np.asarray(bk)) or np.any(np.asarray(bv)))
    nc = _get_nc(bias_free)
    in_maps = shard_inputs(x, Wq, bq, Wk, bk, Wv, bv, Wo)
    res = run_bass_kernel_spmd(nc, in_maps, list(range(NCORES)))
    parts = [res.results[i]["y"] for i in range(NCORES)]
    out = np.stack([parts[2 * b] + parts[2 * b + 1] for b in range(B)])
    out += np.asarray(bo, dtype=np.float32)
    return out.astype(np.float32)


def oracle_core(inp, S_=S):
    """Numpy mirror of the per-core computation, for debugging."""
    def up(t):
        return inp[t].astype(np.float64).transpose(1, 0, 2)

    x8f = up("x8").reshape(DM, S_)
    xr8f = up("xr8").reshape(DM, S_)
    wq8_, wqr8_ = up("wq8").reshape(DM, DLOC), up("wqr8").reshape(DM, DLOC)
    wv8_, wvr8_ = up("wv8").reshape(DM, DLOC), up("wvr8").reshape(DM, DLOC)
    wo8_, wor8_ = up("wo8").reshape(DLOC, DM), up("wor8").reshape(DLOC, DM)
    wk8_ = up("wk8").reshape(DM, DLOC)
    bq_ = inp["bqT"].astype(np.float64).T.reshape(DLOC)
    Q = (x8f.T @ wq8_ + xr8f.T @ wq8_ + x8f.T @ wqr8_) / WS + bq_
    K = (x8f.T @ wk8_) / WS
    V = (x8f.T @ wv8_ + xr8f.T @ wv8_ + x8f.T @ wvr8_) / WS \
        + inp["bvb"][0].astype(np.float64) / WS
    ekb_ = inp["ekb"][0].astype(np.float64)
    out = np.zeros((S_, DLOC))
    for h in range(HLOC):
        slh = slice(h * HD, (h + 1) * HD)
        eq, ek = np.exp(Q[:, slh]), np.exp(K[:, slh]) * ekb_[slh]
        qh = eq / eq.sum(-1, keepdims=True)
        kh = ek / ek.sum(-1, keepdims=True)
        out[:, slh] = qh @ (kh.T @ V[:, slh])
    import ml_dtypes
    f8 = ml_dtypes.float8_e4m3
    o8 = out.astype(f8).astype(np.float64)
    orr = (out - o8).astype(f8).astype(np.float64)
    return ((o8 @ wo8_ + orr @ wo8_ + o8 @ wor8_) / WS).astype(np.float32)


# revision 47
# speedup vs baseline: 1.0351x; 1.0351x over previous
"""Linear-attention Trainium2 kernel (8 NeuronCores, SPMD).

Sharding: batch (4) x head-group (2). Core i handles batch i//2, heads
[8*(i%2), 8*(i%2)+8). Each core computes its partial output through Wo;
the host sums the two partials per batch and adds bo.

Precision scheme (f32 PSUM accumulation everywhere):
  - K projection: 1-term fp8 e4m3 DoubleRow (x8 . wk8)
  - Q, V, output projections: 3-term split-fp8 DoubleRow
      a.w ~= a8.w8 + ar8.w8 + a8.wr8   (ar8/wr8 = fp8 residuals)
    which costs 0.75x one bf16 GEMM and is ~2x more accurate than bf16.
  - Weights are pre-scaled by 16 on the host so their fp8 residuals clear
    the e4m3 denormal floor; the scale is compensated for free in the
    ACT exp (scale=1/16), the out^T copy, and the final y copy.
  - Attention intermediates (ek, vn, eqn, kvsb) are bf16.

Per-core dataflow:
  phase 1 (K/V/KV): per 128-row sequence tile
    K = x8 @ wk8 (DR), ek = exp(K/16) [* exp(bk)]
    rk = 1/rowsum_per_head(ek)
    vn = (x.wv split-fp8 [+ 16*bv]) * rk          [128, 512], 16x scale
    KV_h += ek_h^T @ vn_h  (exact block placement, one memset bank,
                            start=False accumulation, emitted 2 tiles late)
  phase 2 (Q/out/y): per 512-column chunk, interleaved so chunk c's Q
    matmuls cover chunk c-1's out^T and chunk c-2's y-store latencies:
    Q^T = wq.x split-fp8 (DR), eq = exp(Q^T/16 + bq)
    sqb = blockdiag-ones @ eq -> per-head colsums broadcast
    eqn = eq * (1/sqb)
    ops = KV^T-contract eqn;  otc8 = fp8(ops/16), otr8 = fp8 residual
    y = otc.wo split-fp8 (DR), ysb = yps/16, DMA per 512-col half
"""

import numpy as np

B, S, DM, H = 4, 4096, 1024, 16
HD = 64
GROUPS = 2
DLOC = DM // GROUPS   # 512 channels per core
HLOC = H // GROUPS    # 8 heads per core
NCORES = B * GROUPS   # 8
SC = 512              # sequence chunk
NTF = DM // 128       # 8 contraction tiles over d_model
NPAIR = DLOC // 128   # 4 pair-tiles (2 heads each)
WS = 16.0             # host-side weight scale (power of 2)


def make_sel():
    """[128,128] block 'ones' that broadcast per-head colsums: out[q,s] =
    sum_p sel[p,q] eq[p,s] with sel[p,q]=1 iff p,q in the same 64-half."""
    sel = np.zeros((128, 128), np.float32)
    sel[:64, :64] = 1.0
    sel[64:, 64:] = 1.0
    return sel


def build_bass(S_=S, n_devices=NCORES, repeat=1, dbg=False, bias_free=True):
    from contextlib import ExitStack
    import concourse.bass as bass
    import concourse.bacc as bacc
    import concourse.mybir as mybir
    import concourse.tile as tile

    f32 = mybir.dt.float32
    bf16 = mybir.dt.bfloat16
    fp8 = mybir.dt.float8e4
    Exp = mybir.ActivationFunctionType.Exp
    X = mybir.AxisListType.X
    Mult = mybir.AluOpType.mult
    Add = mybir.AluOpType.add
    Sub = mybir.AluOpType.subtract
    DR = mybir.MatmulPerfMode.DoubleRow
    RS = 1.0 / WS

    NCH = S_ // SC        # sequence chunks
    NST = S_ // 128       # sequence tiles

    nc = bacc.Bacc("TRN2", target_bir_lowering=False, debug=False,
                   num_devices=n_devices)
    x8 = nc.dram_tensor("x8", [128, NTF, S_], fp8, kind="ExternalInput").ap()
    xr8 = nc.dram_tensor("xr8", [128, NTF, S_], fp8,
                         kind="ExternalInput").ap()
    wq8 = nc.dram_tensor("wq8", [128, NTF, DLOC], fp8,
                         kind="ExternalInput").ap()
    wqr8 = nc.dram_tensor("wqr8", [128, NTF, DLOC], fp8,
                          kind="ExternalInput").ap()
    wk8 = nc.dram_tensor("wk8", [128, NTF, DLOC], fp8,
                         kind="ExternalInput").ap()
    wv8 = nc.dram_tensor("wv8", [128, NTF, DLOC], fp8,
                         kind="ExternalInput").ap()
    wvr8 = nc.dram_tensor("wvr8", [128, NTF, DLOC], fp8,
                          kind="ExternalInput").ap()
    wo8 = nc.dram_tensor("wo8", [128, NPAIR, DM], fp8,
                         kind="ExternalInput").ap()
    wor8 = nc.dram_tensor("wor8", [128, NPAIR, DM], fp8,
                          kind="ExternalInput").ap()
    bqT = nc.dram_tensor("bqT", [128, NPAIR], f32, kind="ExternalInput").ap()
    ekb = nc.dram_tensor("ekb", [128, DLOC], bf16, kind="ExternalInput").ap()
    bvb = nc.dram_tensor("bvb", [128, DLOC], bf16, kind="ExternalInput").ap()
    sel = nc.dram_tensor("sel", [128, 128], bf16, kind="ExternalInput").ap()
    y = nc.dram_tensor("y", [S_, DM], bf16,
                   kind="ExternalOutput").ap()
    if dbg:
        d_kv = nc.dram_tensor("d_kv", [128, 512], f32,
                              kind="ExternalOutput").ap()

    def body(tc):
        ctx = ExitStack()
        with ctx:
            cons = ctx.enter_context(tc.tile_pool(name="cons", bufs=1))
            xpool = ctx.enter_context(tc.tile_pool(name="xp", bufs=1))
            wpool = ctx.enter_context(tc.tile_pool(name="wp", bufs=1))
            kvsbp = ctx.enter_context(tc.tile_pool(name="kvsb", bufs=1))

            wscr = cons.tile([128, SC], bf16)
            nc.vector.memset(wscr, 0.0)

            # ---- streaming loads, ordered by first use ----
            x8sb = xpool.tile([128, NTF, S_], fp8, tag="x8")
            xr8sb = xpool.tile([128, NTF, S_], fp8, tag="xr8")
            wk_sb = wpool.tile([128, NTF, DLOC], fp8, tag="wk")
            wv_sb = wpool.tile([128, NTF, DLOC], fp8, tag="wv")
            wvr_sb = wpool.tile([128, NTF, DLOC], fp8, tag="wvr")
            wq_sb = wpool.tile([128, NTF, DLOC], fp8, tag="wq")
            wqr_sb = wpool.tile([128, NTF, DLOC], fp8, tag="wqr")
            wo_sb = wpool.tile([128, NPAIR, DM], fp8, tag="wo")
            wor_sb = wpool.tile([128, NPAIR, DM], fp8, tag="wor")
            h1, h2 = slice(0, 4), slice(4, 8)
            nc.sync.dma_start(out=wk_sb[:, h1, :], in_=wk8[:, h1, :])
            nc.sync.dma_start(out=x8sb[:, h1, 0:SC], in_=x8[:, h1, 0:SC])
            nc.sync.dma_start(out=wk_sb[:, h2, :], in_=wk8[:, h2, :])
            nc.sync.dma_start(out=x8sb[:, h2, 0:SC], in_=x8[:, h2, 0:SC])
            nc.sync.dma_start(out=wv_sb[:, h1, :], in_=wv8[:, h1, :])
            nc.sync.dma_start(out=wv_sb[:, h2, :], in_=wv8[:, h2, :])
            nc.sync.dma_start(out=xr8sb[:, :, 0:SC], in_=xr8[:, :, 0:SC])
            nc.sync.dma_start(out=wvr_sb, in_=wvr8)
            nc.sync.dma_start(out=x8sb[:, :, SC:2 * SC],
                              in_=x8[:, :, SC:2 * SC])
            nc.sync.dma_start(out=xr8sb[:, :, SC:2 * SC],
                              in_=xr8[:, :, SC:2 * SC])
            sel_sb = cons.tile([128, 128], bf16)
            nc.sync.dma_start(out=sel_sb, in_=sel)
            bqT_sb = cons.tile([128, NPAIR], f32)
            nc.sync.dma_start(out=bqT_sb, in_=bqT)
            if not bias_free:
                ekb_sb = cons.tile([128, DLOC], bf16)
                nc.sync.dma_start(out=ekb_sb, in_=ekb)
                bvb_sb = cons.tile([128, DLOC], bf16)
                nc.sync.dma_start(out=bvb_sb, in_=bvb)
            for c in range(2, NCH):
                nc.sync.dma_start(out=x8sb[:, :, c * SC:(c + 1) * SC],
                                  in_=x8[:, :, c * SC:(c + 1) * SC])
                nc.sync.dma_start(out=xr8sb[:, :, c * SC:(c + 1) * SC],
                                  in_=xr8[:, :, c * SC:(c + 1) * SC])
            nc.sync.dma_start(out=wq_sb, in_=wq8)
            nc.sync.dma_start(out=wqr_sb, in_=wqr8)
            nc.sync.dma_start(out=wo_sb, in_=wo8)
            nc.sync.dma_start(out=wor_sb, in_=wor8)

            kvsb = kvsbp.tile([128, 512], bf16)
            VT3 = [(x8sb, wv_sb), (xr8sb, wv_sb), (x8sb, wvr_sb)]
            QT3 = [(wq_sb, x8sb), (wq_sb, xr8sb), (wqr_sb, x8sb)]

            # Q-side pools + helpers shared by the phase-1 tail (chunk-0
            # Q prologue) and phase 2. `late` carries the phase-2 psum
            # pools once they exist.
            eqpool = ctx.enter_context(tc.tile_pool(name="eq", bufs=4))
            eqnpool = ctx.enter_context(tc.tile_pool(name="eqn", bufs=2))
            rqpool = ctx.enter_context(tc.tile_pool(name="rq", bufs=4))
            late = {}

            def q_mms(qps, c, dt):
                for term in range(3):
                    wsrc, asrc = QT3[term]
                    for tfp in range(NTF // 2):
                        nc.tensor.matmul(
                            qps,
                            wsrc[:, 2 * tfp:2 * tfp + 2,
                                 dt * 128:(dt + 1) * 128],
                            asrc[:, 2 * tfp:2 * tfp + 2,
                                 c * SC:(c + 1) * SC],
                            start=(term == 0 and tfp == 0),
                            stop=(term == 2 and tfp == NTF // 2 - 1),
                            perf_mode=DR)

            def q_fin_factory(eq, dt, eqn, poolkey):
                def fin():
                    sq = late[poolkey].tile([128, SC], f32, tag="sq"
                                            if poolkey == "sqpsp" else "ops")
                    nc.tensor.matmul(sq, sel_sb, eq, start=True, stop=True)
                    rq = rqpool.tile([128, SC], bf16, tag="rq")
                    with nc.allow_low_precision(reason="bf16 attn"):
                        nc.vector.reciprocal(rq, sq)
                        nc.vector.tensor_tensor(
                            out=eqn[:, dt, :], in0=eq, in1=rq, op=Mult)
                return fin

            # ---------------- phase 1: K / V / KV ----------------
            with ExitStack() as p1:
                pkv = p1.enter_context(
                    tc.tile_pool(name="pkv", bufs=3, space="PSUM"))
                kvp = p1.enter_context(
                    tc.tile_pool(name="kvps", bufs=1, space="PSUM"))
                ekpool = p1.enter_context(tc.tile_pool(name="ek", bufs=6))
                vnpool = p1.enter_context(tc.tile_pool(name="vn", bufs=6))
                smpool = p1.enter_context(tc.tile_pool(name="sm", bufs=6))

                kv = kvp.tile([128, 512], f32)
                # real HW start=True clears only the written region (not the
                # whole 2KB row the sim models), so zero the bank explicitly
                # and accumulate with start=False throughout.
                nc.vector.memset(kv, 0.0)
                pendings = []

                def emit_kv(ek, vn, st):
                    for h in range(HLOC):
                        p_, hf = h // 2, 64 * (h % 2)
                        nc.tensor.matmul(
                            kv[hf:hf + 64, p_ * 128 + hf:p_ * 128 + hf + 64],
                            ek[:, h * HD:(h + 1) * HD],
                            vn[:, h * HD:(h + 1) * HD],
                            start=False,
                            stop=(st == NST - 1),
                            skip_group_check=True)

                def drain_kv(keep=0):
                    if len(pendings) > keep:
                        emit_kv(*pendings.pop(0))

                def emit_k_mms(kps, st):
                    for tfp in range(NTF // 2):
                        nc.tensor.matmul(
                            kps,
                            x8sb[:, 2 * tfp:2 * tfp + 2,
                                 st * 128:(st + 1) * 128],
                            wk_sb[:, 2 * tfp:2 * tfp + 2, :],
                            start=(tfp == 0), stop=(tfp == NTF // 2 - 1),
                            perf_mode=DR)

                def emit_v_mms(vps, st, term, first, last):
                    a, w = VT3[term]
                    for tfp in range(NTF // 2):
                        nc.tensor.matmul(
                            vps,
                            a[:, 2 * tfp:2 * tfp + 2,
                              st * 128:(st + 1) * 128],
                            w[:, 2 * tfp:2 * tfp + 2, :],
                            start=(first and tfp == 0),
                            stop=(last and tfp == NTF // 2 - 1),
                            perf_mode=DR)

                def k_chain(kps):
                    """exp(K/16) + per-head rowsum reciprocal."""
                    ek0 = ekpool.tile([128, DLOC], bf16, tag="ek0")
                    nc.scalar.activation(ek0, kps, Exp, scale=RS)
                    with nc.allow_low_precision(reason="bf16 attn weights"):
                        if bias_free:
                            ek = ek0
                        else:
                            ek = ekpool.tile([128, DLOC], bf16, tag="ek")
                            nc.vector.tensor_tensor(out=ek, in0=ek0,
                                                    in1=ekb_sb, op=Mult)
                        sk = smpool.tile([128, HLOC], bf16, tag="sk")
                        nc.vector.reduce_sum(
                            sk, ek.rearrange("p (h e) -> p h e", e=HD),
                            axis=X)
                        rk = smpool.tile([128, HLOC], bf16, tag="rk")
                        nc.vector.reciprocal(rk, sk)
                    return ek, rk

                def v_chain(st, ek, rk, vps):
                    """bias + normalize one V tile, queue its KV update."""
                    vn = vnpool.tile([128, DLOC], bf16, tag="vn")
                    rkb = bass.AP(
                        tensor=rk.tensor, offset=rk.offset,
                        ap=[list(rk.ap[0]), [1, HLOC], [0, HD]])
                    with nc.allow_low_precision(reason="bf16 attn weights"):
                        if bias_free:
                            v1 = vps
                        else:
                            v1 = vnpool.tile([128, DLOC], bf16, tag="v1")
                            nc.vector.tensor_tensor(out=v1, in0=vps,
                                                    in1=bvb_sb, op=Add)
                        nc.vector.tensor_tensor(
                            out=vn.rearrange("p (h e) -> p h e", e=HD),
                            in0=v1.rearrange("p (h e) -> p h e", e=HD),
                            in1=rkb, op=Mult)
                    pendings.append((ek, vn, st))

                # --- chunk 0: term/tf-major so the PE can consume each
                # weight/x slice the moment it lands ---
                kps0 = []
                for t in range(4):
                    k0 = pkv.tile([128, DLOC], f32, tag=f"c0_{t}", bufs=1)
                    kps0.append(k0)
                # PE warmup during the initial DMA wait (ramps the clock);
                # overwritten by the first real matmul (start=True).
                for _ in range(4):
                    nc.tensor.matmul(kps0[0], wscr[:, 0:128], wscr,
                                     start=True, stop=True)
                for tfp in range(NTF // 2):
                    for t in range(4):
                        nc.tensor.matmul(
                            kps0[t],
                            x8sb[:, 2 * tfp:2 * tfp + 2,
                                 t * 128:(t + 1) * 128],
                            wk_sb[:, 2 * tfp:2 * tfp + 2, :],
                            start=(tfp == 0), stop=(tfp == NTF // 2 - 1),
                            perf_mode=DR)
                chains0 = [k_chain(kps0[t]) for t in range(4)]
                vps0 = []
                for t in range(4):
                    v0 = pkv.tile([128, DLOC], f32, tag=f"c0_{t}", bufs=1)
                    vps0.append(v0)
                for term in range(3):
                    a, w = VT3[term]
                    for tfp in range(NTF // 2):
                        for t in range(4):
                            nc.tensor.matmul(
                                vps0[t],
                                a[:, 2 * tfp:2 * tfp + 2,
                                  t * 128:(t + 1) * 128],
                                w[:, 2 * tfp:2 * tfp + 2, :],
                                start=(term == 0 and tfp == 0),
                                stop=(term == 2 and tfp == NTF // 2 - 1),
                                perf_mode=DR)
                for t in range(4):
                    v_chain(t, chains0[t][0], chains0[t][1], vps0[t])

                # --- chunks 1..7: tile-major, kv emission trails by 2 ---
                for st in range(4, NST):
                    kps = pkv.tile([128, DLOC], f32, tag="pkv")
                    emit_k_mms(kps, st)
                    drain_kv()
                    ek, rk = k_chain(kps)
                    vps = pkv.tile([128, DLOC], f32, tag="pkv")
                    for term in range(3):
                        emit_v_mms(vps, st, term, term == 0, term == 2)
                    v_chain(st, ek, rk, vps)
                    drain_kv(keep=2)

                # chunk-0 Q prologue interleaved with the final kv drains:
                # the q matmuls (reusing the idle chunk-0 psum banks) give
                # the last kv groups time to see their vn results.
                eqn0 = eqnpool.tile([128, NPAIR, SC], bf16, tag="eqn")
                pre_fins = []
                for dt in range(2):
                    qp0 = pkv.tile([128, SC], f32, tag=f"c0_{dt}", bufs=1)
                    q_mms(qp0, 0, dt)
                    eq = eqpool.tile([128, SC], bf16, tag="eq")
                    nc.scalar.activation(eq, qp0, Exp,
                                         bias=bqT_sb[:, dt:dt + 1], scale=RS)
                    pre_fins.append(q_fin_factory(eq, dt, eqn0, "opsp"))
                    drain_kv()
                while pendings:
                    drain_kv()
                nc.scalar.copy(kvsb, kv)
                if dbg:
                    kvf = ekpool.tile([128, 512], f32, tag="kvf")
                    nc.vector.tensor_copy(kvf, kv)
                    nc.sync.dma_start(out=d_kv, in_=kvf)

            # ---------------- phase 2: Q / out / y ----------------
            with ExitStack() as p2:
                otpool = p2.enter_context(tc.tile_pool(name="ot", bufs=2))
                ysbpool = p2.enter_context(tc.tile_pool(name="ysb", bufs=8))
                qpsp = p2.enter_context(
                    tc.tile_pool(name="qps", bufs=2, space="PSUM"))
                sqpsp = p2.enter_context(
                    tc.tile_pool(name="sqps", bufs=1, space="PSUM"))
                opsp = p2.enter_context(
                    tc.tile_pool(name="ops", bufs=2, space="PSUM"))
                ypsp = p2.enter_context(
                    tc.tile_pool(name="yps", bufs=3, space="PSUM"))
                late["sqpsp"] = sqpsp
                late["opsp"] = opsp

                def emit_q(c, dt, eqn):
                    """12 split-fp8 Q matmuls + exp for pair-tile dt; returns
                    a closure finishing the normalization (emitted later)."""
                    qps = qpsp.tile([128, SC], f32, tag="q")
                    q_mms(qps, c, dt)
                    eq = eqpool.tile([128, SC], bf16, tag="eq")
                    nc.scalar.activation(eq, qps, Exp,
                                         bias=bqT_sb[:, dt:dt + 1], scale=RS)
                    return q_fin_factory(eq, dt, eqn, "sqpsp")

                def emit_ops(eqn, otc8, otr8):
                    for dt in range(NPAIR):
                        ops = opsp.tile([128, SC], f32, tag="ops")
                        nc.tensor.matmul(ops,
                                         kvsb[:, dt * 128:(dt + 1) * 128],
                                         eqn[:, dt, :], start=True, stop=True)
                        with nc.allow_low_precision(reason="fp8 split"):
                            nc.scalar.mul(otc8[:, dt, :], ops, RS)
                            nc.vector.scalar_tensor_tensor(
                                out=otr8[:, dt, :], in0=ops, scalar=RS,
                                in1=otc8[:, dt, :], op0=Mult, op1=Sub)

                def emit_y(otc8, otr8, c, t):
                    row = (c * 4 + t) * 128
                    for jh in range(2):
                        yps = ypsp.tile([128, 512], f32, tag="yps")
                        first = True
                        for lsrc, rsrc in ((otc8, wo_sb), (otr8, wo_sb),
                                           (otc8, wor_sb)):
                            for ctp in range(NPAIR // 2):
                                nc.tensor.matmul(
                                    yps,
                                    lsrc[:, 2 * ctp:2 * ctp + 2,
                                         t * 128:(t + 1) * 128],
                                    rsrc[:, 2 * ctp:2 * ctp + 2,
                                         jh * 512:(jh + 1) * 512],
                                    start=first,
                                    stop=(lsrc is otc8 and rsrc is wor_sb
                                          and ctp == NPAIR // 2 - 1),
                                    perf_mode=DR)
                                first = False
                        ysb = ysbpool.tile([128, 512], bf16, tag="ysb")
                        if jh == 0:
                            nc.scalar.mul(ysb, yps, RS)
                        else:
                            nc.vector.tensor_scalar_mul(ysb, yps, RS)
                        nc.sync.dma_start(
                            out=y[row:row + 128, jh * 512:(jh + 1) * 512],
                            in_=ysb)

                yq = []

                def drain_y():
                    if yq:
                        otc8_, otr8_, c_, t_ = yq.pop(0)
                        emit_y(otc8_, otr8_, c_, t_)

                prev = None   # (eqn, c) of previous chunk
                for c in range(NCH):
                    if c == 0:
                        # dt 0/1 were emitted in the phase-1 tail
                        eqn = eqn0
                        for dt in (2, 3):
                            qps = qpsp.tile([128, SC], f32, tag="q")
                            q_mms(qps, 0, dt)
                            eq = eqpool.tile([128, SC], bf16, tag="eq")
                            nc.scalar.activation(
                                eq, qps, Exp,
                                bias=bqT_sb[:, dt:dt + 1], scale=RS)
                            pre_fins.append(
                                q_fin_factory(eq, dt, eqn, "opsp"))
                            pre_fins.pop(0)()
                        pre_fins.pop(0)()
                        pre_fins.pop(0)()
                        prev = (eqn, c)
                        continue
                    eqn = eqnpool.tile([128, NPAIR, SC], bf16, tag="eqn")
                    fins = [emit_q(c, 0, eqn)]
                    if prev is not None:
                        otc8 = otpool.tile([128, NPAIR, SC], fp8, tag="otc")
                        otr8 = otpool.tile([128, NPAIR, SC], fp8, tag="otr")
                        emit_ops(prev[0], otc8, otr8)
                    fins.append(emit_q(c, 1, eqn))
                    fins.append(emit_q(c, 2, eqn))
                    fins.pop(0)()
                    drain_y()
                    fins.append(emit_q(c, 3, eqn))
                    fins.pop(0)()
                    drain_y()
                    fins.pop(0)()
                    drain_y()
                    drain_y()
                    fins.pop(0)()
                    if prev is not None:
                        for t in range(4):
                            yq.append((otc8, otr8, prev[1], t))
                    prev = (eqn, c)
                # epilogue: last chunk's out/y, interleaved with deferred y
                otc8 = otpool.tile([128, NPAIR, SC], fp8, tag="otc")
                otr8 = otpool.tile([128, NPAIR, SC], fp8, tag="otr")
                drain_y()
                emit_ops(prev[0], otc8, otr8)
                while yq:
                    drain_y()
                for t in range(4):
                    emit_y(otc8, otr8, prev[1], t)

    with tile.TileContext(nc) as tc:
        if repeat == 1:
            body(tc)
        else:
            for _ in range(repeat):
                body(tc)
    nc.compile()
    return nc


def _split8(a, f8):
    """fp8 value + fp8 residual of an array."""
    a8 = a.astype(f8)
    r8 = (a - a8.astype(a.dtype)).astype(f8)
    return np.ascontiguousarray(a8), np.ascontiguousarray(r8)


def shard_inputs(x, Wq, bq, Wk, bk, Wv, bv, Wo, S_=S):
    import ml_dtypes
    bf16 = ml_dtypes.bfloat16
    f8 = ml_dtypes.float8_e4m3
    f = np.float32
    sel = make_sel().astype(bf16)
    x = np.asarray(x, dtype=f)
    in_maps = []
    xt_cache = {}
    for core in range(NCORES):
        b, g = core // GROUPS, core % GROUPS
        sl = slice(g * DLOC, (g + 1) * DLOC)
        if b not in xt_cache:
            xr = np.ascontiguousarray(
                x[b, :S_, :].T.reshape(NTF, 128, S_).transpose(1, 0, 2))
            xt_cache[b] = _split8(xr, f8)
        wq_ = np.asarray(Wq, f)[:, sl].reshape(NTF, 128, DLOC).transpose(
            1, 0, 2) * f(WS)
        wk_ = np.asarray(Wk, f)[:, sl].reshape(NTF, 128, DLOC).transpose(
            1, 0, 2) * f(WS)
        wv_ = np.asarray(Wv, f)[:, sl].reshape(NTF, 128, DLOC).transpose(
            1, 0, 2) * f(WS)
        wo_ = np.asarray(Wo, f)[sl, :].reshape(NPAIR, 128, DM).transpose(
            1, 0, 2) * f(WS)
        wq8_, wqr8_ = _split8(wq_, f8)
        wv8_, wvr8_ = _split8(wv_, f8)
        wo8_, wor8_ = _split8(wo_, f8)
        bqT_ = np.ascontiguousarray(
            np.asarray(bq, f)[sl].reshape(NPAIR, 128).T).astype(f)
        ekb_ = np.broadcast_to(np.exp(np.asarray(bk, f)[sl]),
                               (128, DLOC)).astype(bf16)
        bvb_ = np.broadcast_to(np.asarray(bv, f)[sl] * f(WS),
                               (128, DLOC)).astype(bf16)
        in_maps.append({
            "x8": xt_cache[b][0], "xr8": xt_cache[b][1],
            "wq8": wq8_, "wqr8": wqr8_,
            "wk8": np.ascontiguousarray(wk_.astype(f8)),
            "wv8": wv8_, "wvr8": wvr8_, "wo8": wo8_, "wor8": wor8_,
            "bqT": bqT_, "ekb": np.ascontiguousarray(ekb_),
            "bvb": np.ascontiguousarray(bvb_), "sel": sel,
        })
    return in_maps


_NC_CACHE = {}


def _get_nc(bias_free=True):
    key = f"nc{bias_free}"
    if key not in _NC_CACHE:
        _NC_CACHE[key] = build_bass(bias_free=bias_free)
    return _NC_CACHE[key]


def kernel(x, Wq, bq, Wk, bk, Wv, bv, Wo, bo):
    from concourse.bass_utils import run_bass_kernel_spmd
    bias_free = not (np.any(np.asarray(bk)) or np.any(np.asarray(bv)))
    nc = _get_nc(bias_free)
    in_maps = shard_inputs(x, Wq, bq, Wk, bk, Wv, bv, Wo)
    res = run_bass_kernel_spmd(nc, in_maps, list(range(NCORES)))
    parts = [np.asarray(res.results[i]["y"], dtype=np.float32)
             for i in range(NCORES)]
    out = np.stack([parts[2 * b] + parts[2 * b + 1] for b in range(B)])
    out += np.asarray(bo, dtype=np.float32)
    return out.astype(np.float32)


def oracle_core(inp, S_=S):
    """Numpy mirror of the per-core computation, for debugging."""
    def up(t):
        return inp[t].astype(np.float64).transpose(1, 0, 2)

    x8f = up("x8").reshape(DM, S_)
    xr8f = up("xr8").reshape(DM, S_)
    wq8_, wqr8_ = up("wq8").reshape(DM, DLOC), up("wqr8").reshape(DM, DLOC)
    wv8_, wvr8_ = up("wv8").reshape(DM, DLOC), up("wvr8").reshape(DM, DLOC)
    wo8_, wor8_ = up("wo8").reshape(DLOC, DM), up("wor8").reshape(DLOC, DM)
    wk8_ = up("wk8").reshape(DM, DLOC)
    bq_ = inp["bqT"].astype(np.float64).T.reshape(DLOC)
    Q = (x8f.T @ wq8_ + xr8f.T @ wq8_ + x8f.T @ wqr8_) / WS + bq_
    K = (x8f.T @ wk8_) / WS
    V = (x8f.T @ wv8_ + xr8f.T @ wv8_ + x8f.T @ wvr8_) / WS \
        + inp["bvb"][0].astype(np.float64) / WS
    ekb_ = inp["ekb"][0].astype(np.float64)
    out = np.zeros((S_, DLOC))
    for h in range(HLOC):
        slh = slice(h * HD, (h + 1) * HD)
        eq, ek = np.exp(Q[:, slh]), np.exp(K[:, slh]) * ekb_[slh]
        qh = eq / eq.sum(-1, keepdims=True)
        kh = ek / ek.sum(-1, keepdims=True)
        out[:, slh] = qh @ (kh.T @ V[:, slh])
    import ml_dtypes
    f8 = ml_dtypes.float8_e4m3
    o8 = out.astype(f8).astype(np.float64)
    orr = (out - o8).astype(f8).astype(np.float64)
    return ((o8 @ wo8_ + orr @ wo8_ + o8 @ wor8_) / WS).astype(np.float32)
